# revision 46
# baseline (speedup 1.0000x reference)
"""Causal single-head attention (B=4, S=4096, D=1024, fp32) on 8 TRN2 cores.

Sharding: 8 cores = 4 batches x 2 roles (one SPMD NEFF, role picked by
partition_id), split along the KV axis at SPLIT_KV:
  role A (cores 0-3, batch = pid):     kv [0, SPLIT_KV),  queries [0, S)
      = causal triangle below SPLIT_KV plus a maskless full rectangle for
        queries >= SPLIT_KV
  role B (cores 4-7, batch = pid - 4): kv [SPLIT_KV-256, S), queries
      [SPLIT_KV, S) = shifted causal triangle, plus (two-level shift_q
      rebalance) the top one/two kv chunks of role A's late blocks.
Each core emits UNNORMALIZED softmax numerators O^T[d, q] and denominators
den[q] (no running max is needed: logits/32 are bounded ~|3|); the host
merges partials additively and divides: out = (oA + oB) / (dA + dB).

Key algebraic trick: scores = q k^T = x (Wq^T Wk) x^T. The host folds
M = Wq^T Wk (passed as "wqT"), the kernel computes qM^T = M^T x_q^T, and
contracts it against RAW x_k — the K projection is eliminated entirely
(~80 us of the original tensor work per role-B core).

Per-core pipeline (fp32 PSUM accumulation everywhere):
  1. One pass over streamed xT chunks: v = x Wv (bf16 matmuls) and
     qM^T (bf16); kv-range x chunks and qM^T are quantized to fp8e4m3
     pair tiles (fully split per kv-chunk / per query-block so the
     DoubleRow operands are contiguous).
  2. Per query block: all 8 d-chunks of the score contraction run as 4
     fp8 DoubleRow matmuls (2 d-chunks per pass, ~1.53x bf16 rate);
     additive -1e9 staircase masks on diagonal chunks; exp on ScalarE
     emits P^T (bf16) which feeds the PV matmul directly (bf16);
     denominator accumulated on VectorE and reduced by one ones-column
     matmul per block, emitted AFTER the PV matmuls so it never stalls
     the PE; O^T streamed out per d-chunk.
fp8 e4m3 on both score operands costs ~1.8e-2 rel err total (gate 2e-2),
measured deterministic; V/P/PV stay bf16 (fp8 there would breach the gate).
Startup: per-d-chunk DMAs for the first x chunk + wv (dep tracking is
per-tile; small parallel transfers land ~6us earlier than coalesced), plus
~40 dummy warmup matmuls to hold the PE clock-gate at 8/8 through the
DMA-bound bootstrap. HW exec ~355 us (baseline 469 us).
Output per core is O^T [D, S] (bf16 numerators) + den [1, S] fp32; host
transposes and merges.
"""

import numpy as np
import ml_dtypes

BF16 = ml_dtypes.bfloat16

B, S, D = 4, 4096, 1024
SPLIT_KV = 1408
N_CORES = 8
NEG = -1.0e9

_PROGRAM = None


def _role_blocks(q0, q1, m_block):
    """List of (m_start, m_width) query blocks covering [q0, q1)."""
    blocks = []
    m = q0
    while m < q1:
        blocks.append((m, min(m_block, q1 - m)))
        m += m_block
    return blocks


def _build_role(tc, nc, aps, q0, q1, kv0, kv1, m_block, tag, d=D,
                n_lo_default=None, nhi_override=None, extra_chunks=None):
    from concourse import mybir
    from contextlib import ExitStack

    f32 = mybir.dt.float32
    bf16 = mybir.dt.bfloat16
    f8 = mybir.dt.float8e4
    DR = mybir.MatmulPerfMode.DoubleRow
    Exp = mybir.ActivationFunctionType.Exp
    add_op = mybir.AluOpType.add
    NBF = 0                   # d-chunks 0..NBF-1 of q/k in bf16, rest fp8.
    # NBF=0: all 8 score d-chunks in fp8e4m3 DoubleRow (microbench
    # work/mb.py: 282-306 ns per pair-MM vs 233.6 ns per bf16 MM at N=512).
    # Error budget: measured 1.871e-2 vs the 2e-2 gate (deterministic);
    # NBF=2 gives 1.648e-2 at ~+12 us if margin is ever needed.
    scale = float(1.0 / np.sqrt(np.float32(d)))

    xT, wqT, wvT, masks, oT, den = (
        aps["xT"], aps["wqT"], aps["wvT"], aps["masks"],
        aps["oT"], aps["den"],
    )

    DCH = d // 128            # d-chunks
    q_len = q1 - q0
    kv_len = kv1 - kv0
    n_kv = kv_len // 128      # kv chunks held by this role
    nc0 = kv0 // 128          # global index of first held kv chunk
    blocks = _role_blocks(q0, q1, m_block)

    with ExitStack() as ctx:
        # ---- persistent SBUF: kT, v, qT, masks, ones -------------------
        n8 = (DCH - NBF) // 2
        if NBF:
            kt_pool = ctx.enter_context(
                tc.tile_pool(name=f"kt{tag}", bufs=NBF * (-(-kv_len // 512))))
            qt_pool = ctx.enter_context(
                tc.tile_pool(name=f"qt{tag}", bufs=NBF))
        if n8:
            kt8_pool = ctx.enter_context(
                tc.tile_pool(name=f"kt8{tag}", bufs=n8 * n_kv))
            qt8_pool = ctx.enter_context(
                tc.tile_pool(name=f"qt8{tag}", bufs=n8 * len(blocks)))
        v_pool = ctx.enter_context(tc.tile_pool(name=f"v{tag}", bufs=n_kv))
        misc_pool = ctx.enter_context(tc.tile_pool(name=f"misc{tag}", bufs=1))

        # The score "K" operand is RAW x (scores = (x M) x^T with
        # M = Wq^T Wk folded on the host), so the kv-range x chunks persist
        # in SBUF: bf16 per-512-token-chunk tiles for d-chunks 0..NBF-1 and
        # fp8 pair tiles for the rest. No K projection exists.
        n_kc = -(-kv_len // 512)  # 512-token chunks covering the kv range
        ktc = [[kt_pool.tile([128, 512], bf16, tag="kt", name=f"kt{i}_{c}")
                for c in range(n_kc)] for i in range(NBF)] if NBF else []

        def ktx(j, n):
            # bf16 x slice for global kv chunk n, d-chunk j
            col = n * 128 - kv0
            return ktc[j][col // 512][:, col % 512:col % 512 + 128]

        qt = [qt_pool.tile([128, q_len], bf16, tag="qt", name=f"qt{i}")
              for i in range(NBF)] if NBF else []
        # fp8 tiles are fully split — kt8[p][n]: [128, 2, 128] per kv chunk,
        # qt8[p][b]: [128, 2, 512] per query block — so the DoubleRow matmul
        # operands are contiguous (282 ns/pair-MM vs 306 for sliced big tiles)
        n_blk = len(blocks)
        kt8 = [[kt8_pool.tile([128, 2, 128], f8, tag="kt8",
                              name=f"kt8_{p}_{n}") for n in range(n_kv)]
               for p in range(n8)]
        qt8 = [[qt8_pool.tile([128, 2, m_block], f8, tag="qt8",
                              name=f"qt8_{p}_{b}") for b in range(n_blk)]
               for p in range(n8)]
        v = [v_pool.tile([128, d], bf16, tag="v", name=f"v{i}")
             for i in range(n_kv)]

        masks_sb = misc_pool.tile([128, 4, 512], bf16, tag="masks")
        ones_col = misc_pool.tile([128, 1], bf16, tag="ones_col")
        nc.gpsimd.memset(ones_col[:], 1.0)

        # HAM pre-warm: the first ~10us are DMA-bound (engine-sync preamble +
        # first x/wv transfers), during which the PE clock-gate would idle
        # back to 4/8. 40 dummy matmuls keep the PE busy through part of the
        # bootstrap so real matmuls start warm(er). Results are never read.
        # (Longer warmups delay real work via the in-order tensor queue and
        # measured net-negative.)
        warm_sb = misc_pool.tile([128, 256], bf16, tag="warm_sb")
        nc.vector.memset(warm_sb[:], 0.0)
        with tc.tile_pool(name=f"warm{tag}", bufs=1, space="PSUM") as warm_ps:
            wps = warm_ps.tile([128, 256], f32, tag="warm_ps", name="warm_ps")
            for _ in range(40):
                nc.tensor.matmul(wps[:], warm_sb[:, 0:128], warm_sb[:],
                                 start=True, stop=True)



        # ---- phase 1: fused projections (one pass over x) ---------------
        # Each x tile is loaded ONCE; kT / v are projected for its overlap
        # with the kv range and qT for its overlap with the query range.
        s_lo, s_hi = min(kv0, q0), max(kv1, q1)

        with tc.tile_pool(name=f"xt{tag}", bufs=3) as xt_pool, \
             tc.tile_pool(name=f"xtf{tag}", bufs=1) as xtf_pool, \
             tc.tile_pool(name=f"wkv{tag}", bufs=1) as w_pool, \
             tc.tile_pool(name=f"pps{tag}", bufs=8, space="PSUM") as proj_ps:

            # DMA issue on the sync engine costs ~0.6us each, and a single
            # DMA descriptor streams at only ~23 GB/s (engine-parallelism
            # gives the aggregate BW) — so: small per-j DMAs for the
            # startup-critical first chunk + wv, coalesced transfers after.
            # kv-range chunks land d-chunks 0..NBF-1 directly in the
            # persistent ktc tiles (they ARE the score K operand).
            # wv/first-x as per-d-chunk TILES: dependency tracking is
            # per-tile, so the first v matmul must only wait for its own
            # chunk's transfer, not the whole 2MB.
            wvc = [w_pool.tile([128, d], bf16, tag=f"wv{j}", name=f"wv{j}")
                   for j in range(DCH)]
            wq_sb = w_pool.tile([128, DCH, d], bf16, tag="wq")

            def load_xt(s0, sw, ti, first=False):
                """DMA x chunk [s0, s0+sw); returns per-d-chunk AP list."""
                is_kv = s0 < kv1
                xts = []
                if is_kv:
                    c = (s0 - kv0) // 512
                    for j in range(NBF):
                        nc.sync.dma_start(
                            ktc[j][c][:, :sw],
                            xT[j * 128:(j + 1) * 128, s0:s0 + sw])
                        xts.append(ktc[j][c][:, 0:512])
                    if first:
                        ts = [xtf_pool.tile([128, 512], bf16, tag=f"xtf{j}",
                                            name=f"xtf{j}")
                              for j in range(NBF, DCH)]
                        for j in range(NBF, DCH):
                            nc.sync.dma_start(
                                ts[j - NBF][:, :sw],
                                xT[j * 128:(j + 1) * 128, s0:s0 + sw])
                            # startup: interleave wv d-chunk DMAs (v-proj is
                            # the first tensor work and paces through j)
                            nc.sync.dma_start(
                                wvc[j - NBF][:],
                                wvT[(j - NBF) * 128:(j - NBF + 1) * 128, :])
                        xts += [ts[j - NBF][:, :] for j in range(NBF, DCH)]
                        return xts
                    else:
                        t = xt_pool.tile([128, DCH - NBF, 512], bf16,
                                         tag="xt", name=f"xt_{ti}")
                        nc.sync.dma_start(
                            t[:, :, :sw],
                            xT[NBF * 128:d, s0:s0 + sw].rearrange(
                                "(c p) s -> p c s", p=128))
                    xts += [t[:, j - NBF, :] for j in range(NBF, DCH)]
                else:
                    t = xt_pool.tile([128, DCH, 512], bf16, tag="xtq",
                                     name=f"xtq_{ti}")
                    nc.sync.dma_start(
                        t[:, :, :sw],
                        xT[:, s0:s0 + sw].rearrange("(c p) s -> p c s", p=128))
                    xts = [t[:, j, :] for j in range(DCH)]
                return xts

            sw0 = min(512, s_hi - s_lo)
            first_xts = load_xt(s_lo, sw0, 0, first=True)
            for j in range(DCH - NBF, DCH):  # wv chunks not covered above
                nc.sync.dma_start(
                    wvc[j][:], wvT[j * 128:(j + 1) * 128, :])
            nc.sync.dma_start(
                wq_sb[:], wqT.rearrange("(c p) d -> p c d", p=128))
            nc.sync.dma_start(
                masks_sb[:], masks.rearrange("(a p) m -> p a m", p=128)
            )

            def wchain(wcols, i, xts, lo, w_, out_cb):
                # out_cb consumes ps[:, :w_] = d_out chunk i of tokens
                # [s0+lo, s0+lo+w_)
                ps = proj_ps.tile([128, 512], f32, tag="pps", name="pps")
                for j in range(DCH):
                    nc.tensor.matmul(
                        ps[:, :w_],
                        wcols(j, i),
                        xts[j][:, lo:lo + w_],
                        start=(j == 0), stop=(j == DCH - 1),
                    )
                out_cb(ps)

            def wq_cols(j, i):
                return wq_sb[:, j, i * 128:(i + 1) * 128]

            s0 = s_lo
            ti = 0
            while s0 < s_hi:
                sw = min(512, s_hi - s0)
                xts = first_xts if ti == 0 else load_xt(s0, sw, ti)
                klo, khi = max(s0, kv0) - s0, min(s0 + sw, kv1) - s0
                qlo, qhi = max(s0, q0) - s0, min(s0 + sw, q1) - s0
                if klo < khi:
                    # fp8 copies of the raw x kv columns (the score K operand;
                    # klo/khi are 128-aligned)
                    for p8 in range(n8):
                        for h in range(2):
                            j = NBF + 2 * p8 + h
                            for n in range((s0 + klo - kv0) // 128,
                                           (s0 + khi - kv0) // 128):
                                c0 = n * 128 - (s0 - kv0)
                                nc.scalar.copy(kt8[p8][n][:, h, :],
                                               xts[j][:, c0:c0 + 128])
                    # v[s chunk c, d_out] = sum_j (xT[j, c]).T @ WvT[j, :]
                    for c in range(klo // 128, khi // 128):
                        for h0 in range(0, d, 512):
                            hw_ = min(512, d - h0)
                            ps = proj_ps.tile([128, 512], f32, tag="pps",
                                              name="ppsv")
                            for j in range(DCH):
                                nc.tensor.matmul(
                                    ps[:, :hw_],
                                    xts[j][:, c * 128:(c + 1) * 128],
                                    wvc[j][:, h0:h0 + hw_],
                                    start=(j == 0), stop=(j == DCH - 1),
                                )
                            nc.scalar.copy(
                                v[(s0 + c * 128 - kv0) // 128][:, h0:h0 + hw_],
                                ps[:, :hw_]
                            )
                if qlo < qhi:
                    for i in range(DCH):
                        g0 = s0 + qlo - q0

                        def qt_cb(ps, i=i, g0=g0, qw=qhi - qlo):
                            if i < NBF:
                                nc.scalar.copy(qt[i][:, g0:g0 + qw],
                                               ps[:, :qw])
                            else:
                                p8, h = (i - NBF) // 2, (i - NBF) % 2
                                # per-block split tiles; a projection chunk can
                                # straddle two blocks when q0 is misaligned
                                g = g0
                                while g < g0 + qw:
                                    b = g // m_block
                                    off = g - b * m_block
                                    w = min(m_block - off, g0 + qw - g)
                                    nc.scalar.copy(
                                        qt8[p8][b][:, h, off:off + w],
                                        ps[:, g - g0:g - g0 + w])
                                    g += w

                        wchain(wq_cols, i, xts, qlo, qhi - qlo, qt_cb)
                s0 += sw
                ti += 1

        # ---- phase 2: attention per query block ------------------------
        # kv chunks [nc0, min(kv1, m0+mw)/128). Diagonal chunks (rel >= 0)
        # are clipped to their valid column range [rel, mw) and masked with
        # the -1e9 staircase; chunks fully below the diagonal (incl. all
        # chunks of role A's rectangle blocks, m0 >= kv1) need neither.
        n_chunks_max = max(min(kv1, m0 + w) // 128 - nc0 + 1 for m0, w in blocks)
        with tc.tile_pool(name=f"pt{tag}", bufs=n_chunks_max + 4) as pt_pool, \
             tc.tile_pool(name=f"att{tag}", bufs=2) as att_sb, \
             tc.tile_pool(name=f"ob{tag}", bufs=3) as out_sb, \
             tc.tile_pool(name=f"st{tag}", bufs=4, space="PSUM") as st_ps, \
             tc.tile_pool(name=f"ot{tag}", bufs=2, space="PSUM") as ot_ps, \
             tc.tile_pool(name=f"bc{tag}", bufs=2, space="PSUM") as bc_ps:
            def block_chunks(m0, mw):
                # (n_global, lo, use_mask) per kv chunk of this block; the
                # first entry always covers the full [0, mw) column range
                n_hi = min(kv1, m0 + mw) // 128
                if nhi_override and m0 in nhi_override:
                    n_hi = nhi_override[m0]
                nlo = n_lo_default if n_lo_default is not None else nc0
                ents = [(n, max(n * 128 - m0, 0), n * 128 - m0 >= 0)
                        for n in range(nlo, n_hi)]
                if extra_chunks and m0 in extra_chunks:
                    for n_x, lo_x in extra_chunks[m0]:
                        ents.append((n_x, lo_x, False))
                return ents

            for m0, mw in blocks:
                mloc = m0 - q0
                ents = block_chunks(m0, mw)
                acc = att_sb.tile([128, m_block], f32, tag="acc", name="acc")
                pts = []
                for e, (n, lo, use_mask) in enumerate(ents):
                    st = st_ps.tile([128, m_block], f32, tag="st")
                    for j in range(NBF):
                        nc.tensor.matmul(
                            st[:, lo:mw],
                            ktx(j, n),
                            qt[j][:, mloc + lo:mloc + mw],
                            start=(j == 0), stop=(not kt8 and j == NBF - 1),
                        )
                    bidx = mloc // m_block
                    for p in range(len(kt8)):
                        nc.tensor.matmul(
                            st[:, lo:mw],
                            kt8[p][n - nc0][:],
                            qt8[p][bidx][:, :, lo:mw],
                            start=(NBF == 0 and p == 0),
                            stop=(p == len(kt8) - 1),
                            perf_mode=DR,
                        )
                    if use_mask:
                        rel = n * 128 - m0
                        nc.vector.tensor_tensor(
                            st[:, lo:mw], st[:, lo:mw],
                            masks_sb[:, rel // 128, lo:mw], add_op,
                        )
                    pt = pt_pool.tile([128, m_block], bf16, tag="pt", name="pt")
                    nc.scalar.activation(pt[:, lo:mw], st[:, lo:mw], Exp,
                                         scale=scale)
                    pts.append(pt)
                    # accumulate exp tiles (fp32) for the softmax denominator
                    if e == 0:
                        nc.vector.tensor_copy(acc[:, :mw], pt[:, :mw])
                    else:
                        nc.vector.tensor_add(acc[:, lo:mw], acc[:, lo:mw],
                                             pt[:, lo:mw])
                for dd in range(DCH):
                    ot = ot_ps.tile([128, m_block], f32, tag="ot")
                    for e, (n, lo, _) in enumerate(ents):
                        nc.tensor.matmul(
                            ot[:, lo:mw],
                            v[n - nc0][:, dd * 128:(dd + 1) * 128],
                            pts[e][:, lo:mw],
                            start=(e == 0), stop=(e == len(ents) - 1),
                        )
                    o = out_sb.tile([128, m_block], bf16, tag="o")
                    nc.vector.tensor_copy(o[:, :mw], ot[:, :mw])
                    nc.sync.dma_start(
                        oT[dd * 128:(dd + 1) * 128, m0:m0 + mw], o[:, :mw]
                    )
                # denominator = partition-sum of acc via one bf16 ones-matmul
                # (per-partition bf16 rounding errors average out in the sum).
                # Emitted AFTER the PV matmuls: the vector-engine acc chain
                # finishes during PV, so this never stalls the tensor engine.
                accb = att_sb.tile([128, m_block], bf16, tag="accb", name="accb")
                nc.vector.tensor_copy(accb[:, :mw], acc[:, :mw])
                dn_ps = bc_ps.tile([1, m_block], f32, tag="dnp", name="dn_ps")
                nc.tensor.matmul(
                    dn_ps[:, :mw], ones_col[:], accb[:, :mw],
                    start=True, stop=True,
                )
                dsb = att_sb.tile([1, m_block], f32, tag="dsb", name="dsb")
                nc.scalar.copy(dsb[:, :mw], dn_ps[:, :mw])
                nc.sync.dma_start(den[0:1, m0:m0 + mw], dsb[:, :mw])


def build_program(s=S, d=D, split=SPLIT_KV, m_block=512, n_cores=N_CORES):
    """Build and compile the SPMD Bass program. Returns the Bacc object."""
    import concourse.tile as tile
    from concourse import bacc, mybir

    nc = bacc.Bacc(
        "TRN2",
        target_bir_lowering=False,
        debug=False,
        enable_asserts=False,
        num_devices=n_cores,
    )
    bf16 = mybir.dt.bfloat16
    f32 = mybir.dt.float32
    aps = {
        "xT": nc.dram_tensor("xT", [d, s], bf16, kind="ExternalInput").ap(),
        "wqT": nc.dram_tensor("wqT", [d, d], bf16, kind="ExternalInput").ap(),
        "wvT": nc.dram_tensor("wvT", [d, d], bf16, kind="ExternalInput").ap(),
        "masks": nc.dram_tensor("masks", [512, 512], bf16, kind="ExternalInput").ap(),
        "oT": nc.dram_tensor("oT", [d, s], bf16, kind="ExternalOutput").ap(),
        "den": nc.dram_tensor("den", [1, s], f32, kind="ExternalOutput").ap(),
    }
    # Fine-grained A<->B rebalance, two levels: role A's blocks with
    # m0 >= shift_q drop their top kv chunk (split//128 - 1) and blocks
    # with m0 >= shift_q2 also drop the chunk below; role B overlaps kv
    # by two chunks and picks them up for the affected queries
    # (column-clipped, maskless — all shifted queries are > the chunks).
    shift_q = 1536
    shift_q2 = 3072
    n_sh = split // 128 - 1
    a_nhi = {m0: (n_sh - 1 if m0 >= shift_q2 else n_sh)
             for m0, _ in _role_blocks(0, s, m_block) if m0 >= shift_q}
    b_kv0 = split - 256
    b_extra = {}
    for m0, mw in _role_blocks(split, s, m_block):
        ex = []
        if m0 + mw > shift_q:
            ex.append((n_sh, max(shift_q - m0, 0)))
        if m0 + mw > shift_q2:
            ex.append((n_sh - 1, max(shift_q2 - m0, 0)))
        if ex:
            b_extra[m0] = ex
    with tile.TileContext(nc) as tc:
        pid = nc.partition_id()
        with tc.If(pid < n_cores // 2) as cmp:
            _build_role(tc, nc, aps, 0, s, 0, split, m_block, "a", d=d,
                        nhi_override=a_nhi)
        with cmp.Else():
            _build_role(tc, nc, aps, split, s, b_kv0, s, m_block, "b", d=d,
                        n_lo_default=split // 128, extra_chunks=b_extra)
    nc.compile()
    return nc


def host_masks():
    part = np.arange(128, dtype=np.int64)[:, None]
    col = np.arange(512, dtype=np.int64)[None, :]
    m = np.zeros((4, 128, 512), np.float32)
    for r in range(4):
        m[r] = np.where(col >= part + r * 128, 0.0, NEG)
    return np.ascontiguousarray(m.reshape(512, 512).astype(BF16))


def make_in_maps(x, Wq, Wk, Wv):
    # scores = q k^T = x (Wq^T Wk) x^T: fold Wq/Wk into M and contract
    # (x M) against raw x — the K projection disappears entirely.
    M = (Wq.T.astype(np.float32) @ Wk.astype(np.float32))
    wqT = np.ascontiguousarray(M.astype(BF16))
    wvT = np.ascontiguousarray(Wv.T.astype(BF16))
    masks = host_masks()
    xT = np.ascontiguousarray(x.astype(BF16).transpose(0, 2, 1))  # [B, D, S]
    in_maps = []
    for c in range(N_CORES):
        b = c % B
        in_maps.append({
            "xT": xT[b], "wqT": wqT, "wvT": wvT, "masks": masks,
        })
    return in_maps


def gather_output(results):
    out = np.empty((B, S, D), np.float32)
    for b in range(B):
        # role B wrote only queries >= SPLIT_KV; its buffers are
        # zero-initialized elsewhere, so plain addition merges the partials
        num = (results[b]["oT"].astype(np.float32)
               + results[B + b]["oT"].astype(np.float32))      # [D, S]
        dsum = results[b]["den"] + results[B + b]["den"]       # [1, S]
        out[b] = (num / dsum).T
    return out


def get_program():
    global _PROGRAM
    if _PROGRAM is None:
        _PROGRAM = build_program()
    return _PROGRAM


def kernel(x, Wq, Wk, Wv, _trace=False, _trace_cores=None):
    from concourse import bass_utils

    nc = get_program()
    in_maps = make_in_maps(x, Wq, Wk, Wv)
    res = bass_utils.run_bass_kernel_spmd(
        nc, in_maps, core_ids=list(range(N_CORES)),
        trace=_trace, trace_cores=_trace_cores,
    )
    out = gather_output(res.results)
    if _trace:
        kernel.last_results = res
    return out

